# revision 120
# baseline (speedup 1.0000x reference)
"""Trainium2 Bass kernel: pre-LN transformer decoder layer on 8 NeuronCores.

Sharding: core = 4*b + g  (b in {0,1} batch, g in {0..3} group rank).
  - Attention: head-parallel (4 of 16 heads per core) over the full batch-b
    sequence, computed in a TRANSPOSED-score formulation:
      scores_T[k, q] = K_blk @ Q^T   (softmax needs no max-subtraction here;
      scores are O(6)), exp on ScalarE with the key-padding mask as a
      per-partition bias, denominator via an appended ones-column on V,
      so no P-transposes and no reductions at all.  attn output comes out
      already transposed [dh, q] — perfect for the output projection.
  - Output projection partials (4 local heads) are computed per 512-token
    group and exchanged with chunked bf16 ReduceScatters (1 MB each),
    overlapped with the attention compute of later token groups.
  - FFN: token-parallel (512 tokens per core, strided 128-blocks) with
    full weights (bf16).
All matmul operands are bf16 (fp32 accumulation in PSUM).
"""
import math
import numpy as np
import ml_dtypes

import concourse.bacc as bacc
import concourse.bass as bass
import concourse.tile as tile
from concourse import mybir
from concourse.masks import make_identity

B, S, D, H, DH, DFF = 2, 2048, 1024, 16, 64, 4096
G = 4            # cores per batch
LH = H // G      # local heads
LD = LH * DH     # 256 local head dims
SL = S // G      # 512 tokens per core for FFN / output projection
P = 128
NB = S // P      # 16 token blocks
DC = D // P      # 8 d chunks
F32 = mybir.dt.float32
F32R = mybir.dt.float32r
BF16 = mybir.dt.bfloat16
NEG = -1e9
NPBF16 = ml_dtypes.bfloat16

_CACHE = {}


def r(ap):
    return ap.bitcast(F32R)


def build_nc():
    nc = bacc.Bacc("TRN2", target_bir_lowering=False, debug=False, num_devices=8)
    d = {}
    def inp(name, shape, dt=F32):
        d[name] = nc.dram_tensor(name, list(shape), dt, kind="ExternalInput").ap()
    inp("xfull", (S, D))
    inp("xrows", (SL, D))
    inp("wqT", (D, LD), BF16); inp("wkT", (D, LD), BF16); inp("wvT", (D, LD), BF16)
    inp("qb", (P, 2)); inp("kb2", (P, 2)); inp("vb_row", (1, LD), BF16)
    inp("wo_pair", (P, 2, D), BF16); inp("wob_bc", (P, D))
    inp("mdiagT", (P, P)); inp("rmask16", (P, NB)); inp("qm16", (P, NB))

    inp("w1T", (D, DFF), BF16); inp("b1p", (P, DFF // P))
    inp("w2T", (DFF, D), BF16); inp("b2_bc", (P, D))
    out_rows = nc.dram_tensor("out_rows", [SL, D], F32, kind="ExternalOutput").ap()
    partial_d = nc.dram_tensor("partial_d", [S, D], BF16).ap()
    rs_d = nc.dram_tensor("rs_d", [SL, D], BF16).ap()

    with tile.TileContext(nc) as tc:
        with tc.tile_pool(name="consts", bufs=1) as consts:
            identb = consts.tile([P, P], BF16)
            make_identity(nc, identb)
            eps_sb = consts.tile([P, 1], F32)
            nc.vector.memset(eps_sb, 1e-5)
            ones1v = consts.tile([1, P], BF16)
            nc.vector.memset(ones1v, 1.0)
            w1_cm = tc.tile_pool(name="w1p", bufs=1)
            w1_pool = w1_cm.__enter__()
            w1a = w1_pool.tile([P, DC, DFF // 2], BF16)   # first half prefetched
            pw_cm = tc.tile_pool(name="pw", bufs=1)
            pw = pw_cm.__enter__()
            wo_sb = pw.tile([P, 2, D], BF16)
            mdT = pw.tile([P, P], F32)
            rm_sb = pw.tile([P, NB], F32)
            xr_sb = pw.tile([P, 4, D], F32)
            wob = pw.tile([P, D], F32)
            b1_sb = pw.tile([P, DFF // P], F32)
            b2_sb = pw.tile([P, D], F32)
            XP = pw.tile([P, 4, D], F32)   # X' rows (post-attn residual)
            YNT = pw.tile([P, DC, SL], BF16)
            dpst_cm = tc.tile_pool(name="d_ps_t", bufs=2, space="PSUM")
            d_ps_t = dpst_cm.__enter__()
            qt_cm = tc.tile_pool(name="qt", bufs=1)
            qt_pool = qt_cm.__enter__()
            QT = qt_pool.tile([P, 2, S], BF16)      # [dh-in-pair, pb, s] (q scaled+bias)
            KT = qt_pool.tile([P, 2, S], BF16)
            V = qt_pool.tile([P, NB, LH, DH + 1], BF16)   # token-major V + ones col
            nc.vector.memset(V[:, :, :, DH:DH + 1], 1.0)

            # ---------------- Phase A: LN1 + transpose, Phase B: QKV ----------
            with tc.tile_pool(name="ab", bufs=4) as ab, \
                 tc.tile_pool(name="abw", bufs=1) as abw, \
                 tc.tile_pool(name="xnt_p", bufs=1) as xnt_p, \
                 tc.tile_pool(name="qkps", bufs=2, space="PSUM") as qkps:
                wq_sb = abw.tile([P, DC, LD], BF16)
                wk_sb = abw.tile([P, DC, LD], BF16)
                wv_sb = abw.tile([P, DC, LD], BF16)
                qb_sb = abw.tile([P, 2], F32); nc.sync.dma_start(out=qb_sb, in_=d["qb"][:])
                kb_sb = abw.tile([P, 2], F32); nc.sync.dma_start(out=kb_sb, in_=d["kb2"][:])
                vbr_sb = abw.tile([1, LD], BF16); nc.sync.dma_start(out=vbr_sb, in_=d["vb_row"][:])
                qm_sb = abw.tile([P, NB], F32); nc.sync.dma_start(out=qm_sb, in_=d["qm16"][:])
                XNT = xnt_p.tile([P, DC, S], BF16)

                for i in range(NB):
                    xin = ab.tile([P, D], F32, tag="xin")
                    nc.sync.dma_start(out=xin, in_=d["xfull"][i * P:(i + 1) * P, :])
                    stats = ab.tile([P, 2, 6], F32, tag="st")
                    nc.vector.bn_stats(out=stats[:, 0, :], in_=xin[:, 0:512])
                    nc.vector.bn_stats(out=stats[:, 1, :], in_=xin[:, 512:1024])
                    mv = ab.tile([P, 2], F32, tag="mv")
                    nc.vector.bn_aggr(out=mv, in_=stats)
                    rs_t = ab.tile([P, 1], F32, tag="rs")
                    nc.scalar.activation(out=rs_t, in_=mv[:, 1:2],
                                         func=mybir.ActivationFunctionType.Sqrt,
                                         bias=eps_sb)
                    nc.vector.reciprocal(out=rs_t, in_=rs_t)
                    # zero padded-token rows of Xn: their q columns become
                    # exactly 0 (folded q-bias is 0 too), so exp gives uniform
                    # weights over valid causal keys — matching the reference's
                    # fp32-quantized -1e9 masking on padded query rows
                    nc.vector.tensor_tensor(out=rs_t, in0=rs_t,
                                            in1=qm_sb[:, i:i + 1],
                                            op=mybir.AluOpType.mult)
                    nmurs = ab.tile([P, 1], F32, tag="nm")
                    nc.vector.tensor_tensor(out=nmurs, in0=mv[:, 0:1], in1=rs_t,
                                            op=mybir.AluOpType.mult)
                    nc.vector.tensor_scalar_mul(nmurs, nmurs, -1.0)
                    xnb = ab.tile([P, D], BF16, tag="xnb")
                    nc.scalar.activation(out=xnb, in_=xin,
                                         func=mybir.ActivationFunctionType.Identity,
                                         scale=rs_t, bias=nmurs)
                    for j in range(DC // 2):
                        pt = d_ps_t.tile([P, 2, P], BF16, tag="tp3")
                        nc.tensor.transpose(pt[:, 0, :], xnb[:, 2 * j * P:(2 * j + 1) * P], identb)
                        nc.tensor.transpose(pt[:, 1, :], xnb[:, (2 * j + 1) * P:(2 * j + 2) * P], identb)
                        nc.any.tensor_copy(out=XNT[:, 2 * j:2 * j + 2, i * P:(i + 1) * P],
                                           in_=pt)

                # QKV weights + FFN w1 + all later-phase weights load here, after
                # the input-stream DMAs are issued, so they never delay startup
                nc.sync.dma_start(out=wq_sb, in_=d["wqT"].rearrange("(c p) o -> p c o", p=P))
                nc.sync.dma_start(out=wk_sb, in_=d["wkT"].rearrange("(c p) o -> p c o", p=P))
                nc.sync.dma_start(out=wv_sb, in_=d["wvT"].rearrange("(c p) o -> p c o", p=P))
                w1rr = d["w1T"].rearrange("(c p) m -> p c m", p=P)
                for c in range(DC):
                    nc.sync.dma_start(out=w1a[:, c, :], in_=w1rr[:, c, 0:DFF // 2])
                nc.sync.dma_start(out=wo_sb, in_=d["wo_pair"][:])
                nc.sync.dma_start(out=mdT, in_=d["mdiagT"][:])
                nc.sync.dma_start(out=rm_sb, in_=d["rmask16"][:])
                nc.sync.dma_start(out=xr_sb, in_=d["xrows"].rearrange("(s p) d -> p s d", p=P))
                nc.sync.dma_start(out=wob, in_=d["wob_bc"][:])
                nc.sync.dma_start(out=b1_sb, in_=d["b1p"][:])
                nc.sync.dma_start(out=b2_sb, in_=d["b2_bc"][:])

                # QKV projections
                for pb in range(2):
                    for sc in range(S // 512):
                        psq = qkps.tile([P, 512], F32, tag="psq")
                        psk = qkps.tile([P, 512], F32, tag="psk")
                        for dc in range(DC):
                            nc.tensor.matmul(psq, wq_sb[:, dc, pb * P:(pb + 1) * P],
                                             XNT[:, dc, sc * 512:(sc + 1) * 512],
                                             start=(dc == 0), stop=(dc == DC - 1))
                        for dc in range(DC):
                            nc.tensor.matmul(psk, wk_sb[:, dc, pb * P:(pb + 1) * P],
                                             XNT[:, dc, sc * 512:(sc + 1) * 512],
                                             start=(dc == 0), stop=(dc == DC - 1))
                        nc.scalar.activation(out=QT[:, pb, sc * 512:(sc + 1) * 512],
                                             in_=psq,
                                             func=mybir.ActivationFunctionType.Identity,
                                             bias=qb_sb[:, pb:pb + 1])

                        nc.scalar.activation(out=KT[:, pb, sc * 512:(sc + 1) * 512],
                                             in_=psk,
                                             func=mybir.ActivationFunctionType.Identity,
                                             bias=kb_sb[:, pb:pb + 1])
                for sb in range(NB):
                    psv = qkps.tile([P, LD], F32, tag="psv")
                    # rank-1 bias init, so the V write is a plain copy below
                    nc.tensor.matmul(psv, ones1v, vbr_sb, start=True, stop=False)
                    for dc in range(DC):
                        nc.tensor.matmul(psv, XNT[:, dc, sb * P:(sb + 1) * P],
                                         wv_sb[:, dc, :],
                                         start=False, stop=(dc == DC - 1))
                    nc.any.tensor_copy(
                        out=V[:, sb, :, 0:DH],
                        in_=psv[:].rearrange("p (h e) -> p h e", h=LH))

            # ---------------- Phase C: attention (transposed scores) ----------
            with tc.tile_pool(name="cat", bufs=2) as cat, \
                 tc.tile_pool(name="cee", bufs=3) as cee, \
                 tc.tile_pool(name="csm", bufs=2) as csm, \
                 tc.tile_pool(name="c_ps_s", bufs=2, space="PSUM") as c_ps_s, \
                 tc.tile_pool(name="c_ps_at", bufs=1, space="PSUM") as c_ps_at:
                ATgs = {}
                ynbs = {}

                def emit_wo_partials(g):
                    # output-projection partials for token group g (bf16) + RS
                    for qi2 in range(4):
                        row0 = (g * 4 + qi2) * P
                        for oc in range(2):
                            pp = c_ps_s.tile([P, 512], F32, tag="ps0", name="pp")
                            for hp in range(2):
                                nc.tensor.matmul(
                                    pp, ATgs[g][:, hp, qi2 * P:(qi2 + 1) * P],
                                    wo_sb[:, hp, oc * 512:(oc + 1) * 512],
                                    start=(hp == 0), stop=(hp == 1))
                            ppb = cee.tile([P, 512], BF16, tag="ppb")
                            nc.any.tensor_copy(out=ppb, in_=pp)
                            nc.sync.dma_start(
                                out=partial_d[row0:row0 + P, oc * 512:(oc + 1) * 512],
                                in_=ppb)
                    nc.gpsimd.collective_compute(
                        "ReduceScatter", mybir.AluOpType.add,
                        replica_groups=[[0, 1, 2, 3], [4, 5, 6, 7]],
                        ins=[partial_d[g * 512:(g + 1) * 512, :]],
                        outs=[rs_d[g * P:(g + 1) * P, :]])

                def emit_d_pro_vec(sb):
                    # residual + LN2 for token block sb (DVE/ScalarE only)
                    rs_sb = csm.tile([P, D], BF16, tag="rs_in")
                    nc.sync.dma_start(out=rs_sb, in_=rs_d[sb * P:(sb + 1) * P, :])
                    nc.vector.tensor_tensor(out=XP[:, sb, :], in0=rs_sb,
                                            in1=xr_sb[:, sb, :],
                                            op=mybir.AluOpType.add)
                    nc.vector.tensor_tensor(out=XP[:, sb, :], in0=XP[:, sb, :],
                                            in1=wob, op=mybir.AluOpType.add)
                    stats = csm.tile([P, 2, 6], F32, tag="st2")
                    nc.vector.bn_stats(out=stats[:, 0, :], in_=XP[:, sb, 0:512])
                    nc.vector.bn_stats(out=stats[:, 1, :], in_=XP[:, sb, 512:1024])
                    mv = csm.tile([P, 2], F32, tag="mv2")
                    nc.vector.bn_aggr(out=mv, in_=stats)
                    rs_t = csm.tile([P, 1], F32, tag="rs2")
                    nc.scalar.activation(out=rs_t, in_=mv[:, 1:2],
                                         func=mybir.ActivationFunctionType.Sqrt,
                                         bias=eps_sb)
                    nc.vector.reciprocal(out=rs_t, in_=rs_t)
                    nmurs = csm.tile([P, 1], F32, tag="nm2")
                    nc.vector.tensor_tensor(out=nmurs, in0=mv[:, 0:1], in1=rs_t,
                                            op=mybir.AluOpType.mult)
                    nc.vector.tensor_scalar_mul(nmurs, nmurs, -1.0)
                    ynb = ynbs[sb] = pw.tile([P, D], BF16, tag=f"ynb{sb}",
                                             name="ynb")
                    nc.scalar.activation(out=ynb, in_=XP[:, sb, :],
                                         func=mybir.ActivationFunctionType.Identity,
                                         scale=rs_t, bias=nmurs)

                def emit_d_pro_tp(sb):
                    # transpose ynb into YNT (tensor engine)
                    for j in range(DC // 2):
                        tp = d_ps_t.tile([P, 2, P], BF16, tag="tp3")
                        nc.tensor.transpose(tp[:, 0, :],
                                            ynbs[sb][:, 2 * j * P:(2 * j + 1) * P],
                                            identb)
                        nc.tensor.transpose(tp[:, 1, :],
                                            ynbs[sb][:, (2 * j + 1) * P:(2 * j + 2) * P],
                                            identb)
                        nc.any.tensor_copy(
                            out=YNT[:, 2 * j:2 * j + 2, sb * P:(sb + 1) * P],
                            in_=tp)

                for gq in range(4):
                    ATg = cat.tile([P, 2, 512], BF16, tag="ATg")
                    ATgs[gq] = ATg
                    for pb in range(2):   # head pair (2*pb, 2*pb+1), row-packed
                        if pb == 1 and gq > 0:
                            # previous group's partials: its ATg is long since
                            # ready, so these don't stall the tensor queue
                            emit_wo_partials(gq - 1)
                        if gq == 3:
                            # residual+LN2 for early token blocks (DVE-only;
                            # their RS chunks completed a group ago)
                            emit_d_pro_vec(pb)
                        at0 = c_ps_at.tile([DH + 1, 512], F32, tag="at0")
                        at1 = c_ps_at.tile([DH + 1, 512], F32, tag="at1")
                        ats = (at0, at1)
                        for kb in range(4 * gq + 4):
                            q0, qe = max(kb, 4 * gq), 4 * gq + 4
                            w = (qe - q0) * P
                            col0 = (q0 - 4 * gq) * P
                            ps0 = c_ps_s.tile([P, 512], F32, tag="ps0")
                            ps1 = c_ps_s.tile([P, 512], F32, tag="ps1")
                            pss = (ps0, ps1)
                            # two heads in different PE row-groups -> concurrent
                            for i, po in enumerate((0, 64)):
                                nc.tensor.matmul(
                                    pss[i][:, :w],
                                    KT[po:po + 64, pb, kb * P:(kb + 1) * P],
                                    QT[po:po + 64, pb, q0 * P:qe * P],
                                    start=True, stop=True)
                            for i in range(2):
                                if q0 == kb:
                                    nc.vector.tensor_tensor(out=pss[i][:, 0:P],
                                                            in0=pss[i][:, 0:P],
                                                            in1=mdT,
                                                            op=mybir.AluOpType.add)
                                ept = cee.tile([P, 512], BF16, tag="ept")
                                nc.scalar.activation(
                                    out=ept[:, :w], in_=pss[i][:, :w],
                                    func=mybir.ActivationFunctionType.Exp,
                                    bias=rm_sb[:, kb:kb + 1])
                                Vst = V[:, kb, 2 * pb + i, :]
                                if q0 == kb:
                                    nc.tensor.matmul(ats[i][:, col0:col0 + P], Vst,
                                                     ept[:, 0:P],
                                                     start=(kb == 0), stop=True)
                                    if w > P:
                                        nc.tensor.matmul(ats[i][:, col0 + P:col0 + w],
                                                         Vst, ept[:, P:w],
                                                         start=(kb == 0), stop=False)
                                else:
                                    nc.tensor.matmul(ats[i][:, 0:w], Vst, ept[:, :w],
                                                     start=(kb == 0), stop=False)
                        for i in range(2):
                            rinv = csm.tile([1, 512], F32, tag="ri")
                            nc.vector.reciprocal(out=rinv, in_=ats[i][DH:DH + 1, :])
                            rb_sb = csm.tile([64, 512], F32, tag="rbs")
                            nc.gpsimd.partition_broadcast(rb_sb, rinv[0:1, :],
                                                          channels=64)
                            nc.vector.tensor_tensor(out=ATg[i * 64:(i + 1) * 64, pb, :],
                                                    in0=ats[i][0:DH, :], in1=rb_sb,
                                                    op=mybir.AluOpType.mult)
                emit_wo_partials(3)
                emit_d_pro_tp(0)
                emit_d_pro_tp(1)
                emit_d_pro_vec(2)
                emit_d_pro_tp(2)
                emit_d_pro_vec(3)

            qt_cm.__exit__(None, None, None)

            # ---------------- Phase D: FFN (LN2 prologue already emitted) -----
            with tc.tile_pool(name="dh", bufs=1) as dh_p, \
                 tc.tile_pool(name="dw1", bufs=3) as dw1, \
                 tc.tile_pool(name="dw2", bufs=6) as dw2_p, \
                 tc.tile_pool(name="dfin", bufs=3) as dfin, \
                 tc.tile_pool(name="d_ps_h", bufs=2, space="PSUM") as d_ps_h, \
                 tc.tile_pool(name="d_ps_o", bufs=1, space="PSUM") as d_ps_o:
                HT = dh_p.tile([P, DFF // P, SL], BF16)

                def w1_half(th):
                    # first token half (sb 0,1) starts while the last RS chunk
                    # is still in flight; upper-half w1 chunks are streamed
                    t0, t1 = th * 256, (th + 1) * 256
                    for c in range(DFF // P):
                        if c < DFF // (2 * P):
                            w1c = [w1a[:, dc, c * P:(c + 1) * P] for dc in range(DC)]
                        else:
                            w1t = dw1.tile([P, DC, P], BF16, tag="w1c")
                            nc.sync.dma_start(out=w1t,
                                              in_=w1rr[:, :, c * P:(c + 1) * P])
                            w1c = [w1t[:, dc, :] for dc in range(DC)]
                        ps_h = d_ps_h.tile([P, 256], F32, tag="ps_h")
                        for dc in range(DC):
                            nc.tensor.matmul(ps_h, w1c[dc], YNT[:, dc, t0:t1],
                                             start=(dc == 0), stop=(dc == DC - 1))
                        nc.scalar.activation(out=HT[:, c, t0:t1], in_=ps_h,
                                             func=mybir.ActivationFunctionType.Gelu,
                                             bias=b1_sb[:, c:c + 1])

                w1_half(0)
                emit_d_pro_tp(3)
                w1_half(1)
                w2r = d["w2T"].rearrange("(c p) o -> p c o", p=P)

                for oc in range(2):
                    ps_os = [d_ps_o.tile([P, 512], F32, tag=f"ps_o{sb}", name=f"ps_o{sb}")
                             for sb in range(4)]
                    for c in range(DFF // P):
                        w2c = dw2_p.tile([P, 512], BF16, tag="w2c")
                        nc.sync.dma_start(out=w2c,
                                          in_=w2r[:, c, oc * 512:(oc + 1) * 512])
                        for sb in range(4):
                            nc.tensor.matmul(
                                ps_os[sb], HT[:, c, sb * P:(sb + 1) * P],
                                w2c,
                                start=(c == 0), stop=(c == DFF // P - 1))
                    for sb in range(4):
                        fin = dfin.tile([P, 512], F32, tag="fin")
                        nc.vector.tensor_tensor(out=fin, in0=ps_os[sb],
                                                in1=b2_sb[:, oc * 512:(oc + 1) * 512],
                                                op=mybir.AluOpType.add)
                        nc.vector.tensor_tensor(out=fin, in0=fin,
                                                in1=XP[:, sb, oc * 512:(oc + 1) * 512],
                                                op=mybir.AluOpType.add)
                        nc.sync.dma_start(
                            out=out_rows[sb * P:(sb + 1) * P, oc * 512:(oc + 1) * 512],
                            in_=fin)

            dpst_cm.__exit__(None, None, None)
            pw_cm.__exit__(None, None, None)
            w1_cm.__exit__(None, None, None)

    nc.compile()
    return nc


def make_in_maps(X, mask, valid_lens, wq_w, wq_b, wk_w, wv_w, wv_b, wo_w, wo_b,
                 ln1_g, ln1_b, ln2_g, ln2_b, w1, b1, w2, b2):
    f = np.float32
    bc = lambda v: np.broadcast_to(np.asarray(v, f)[None, :], (P, len(v))).copy()
    # transposed causal diagonal block (identical for every kb):
    # mdT[kk, qq] = mask[qq, kk] for the block-local causal pattern
    mdT = np.ascontiguousarray(np.asarray(mask[0:P, 0:P]).T).astype(f)
    idx = np.arange(S)
    woT = np.ascontiguousarray(np.asarray(wo_w, f).T)   # [d_in, d_out]
    p_ar = np.arange(P)
    in_maps = []
    for core in range(8):
        b, g = core // G, core % G
        vmask1 = np.where(idx >= valid_lens[b], NEG, 0.0).astype(f)
        hs = slice(g * LD, (g + 1) * LD)
        # local-head-pair layout matching ATg: partition p = (h%2)*64 + dh,
        # second dim hp = h//2, local head h = hp*2 + p//64
        wo_pair = np.empty((P, 2, D), NPBF16)
        for hp in range(2):
            head = g * 4 + hp * 2 + p_ar // 64
            rows = head * 64 + (p_ar % 64)
            wo_pair[:, hp, :] = woT[rows, :].astype(NPBF16)
        # strided token ownership: rows sb*512 + g*128 .. +128, sb = 0..3
        own = np.concatenate([np.arange(sb * 512 + g * P, sb * 512 + (g + 1) * P)
                              for sb in range(4)])
        m = {
            "xfull": np.ascontiguousarray(X[b]).astype(f),
            "xrows": np.ascontiguousarray(X[b][own]).astype(f),
            "wqT": np.ascontiguousarray(
                (np.asarray(wq_w, f)[hs] * np.asarray(ln1_g, f)[None, :] * 0.125)
                .T).astype(NPBF16),
            "wkT": np.ascontiguousarray(
                (np.asarray(wk_w, f)[hs] * np.asarray(ln1_g, f)[None, :])
                .T).astype(NPBF16),
            "wvT": np.ascontiguousarray(
                (np.asarray(wv_w, f)[hs] * np.asarray(ln1_g, f)[None, :])
                .T).astype(NPBF16),
            "qb": np.ascontiguousarray(
                (0.125 * (wq_b[hs] + ln1_b @ np.asarray(wq_w, f)[hs].T))
                .reshape(2, P).T).astype(f),
            "kb2": np.ascontiguousarray(
                (ln1_b @ np.asarray(wk_w, f)[hs].T).reshape(2, P).T).astype(f),
            "vb_row": np.ascontiguousarray(
                (wv_b[hs] + ln1_b @ np.asarray(wv_w, f)[hs].T)
                .reshape(1, LD)).astype(NPBF16),
            "wo_pair": wo_pair,
            "wob_bc": bc(wo_b),
            "mdiagT": mdT,
            "rmask16": np.ascontiguousarray(vmask1.reshape(NB, P).T).astype(f),
            "qm16": np.ascontiguousarray(
                np.where(idx < valid_lens[b], 1.0, 0.0).astype(f)
                .reshape(NB, P).T).astype(f),
            "w1T": np.ascontiguousarray(
                (np.asarray(w1, f) * np.asarray(ln2_g, f)[None, :])
                .T).astype(NPBF16),
            "b1p": np.ascontiguousarray(
                np.asarray(b1 + ln2_b @ np.asarray(w1, f).T, f)
                .reshape(DFF // P, P).T).astype(f),
            "w2T": np.ascontiguousarray(np.asarray(w2, f).T).astype(NPBF16),
            "b2_bc": bc(b2),
        }
        in_maps.append(m)
    return in_maps


def kernel(**inputs):
    from concourse.bass_utils import run_bass_kernel_spmd
    if "nc" not in _CACHE:
        _CACHE["nc"] = build_nc()
    nc = _CACHE["nc"]
    in_maps = make_in_maps(**inputs)
    res = run_bass_kernel_spmd(nc, in_maps, list(range(8)))
    out = np.empty((B, S, D), np.float32)
    for core in range(8):
        b, g = core // G, core % G
        rows = res.results[core]["out_rows"]
        for sb in range(4):
            out[b, sb * 512 + g * P:sb * 512 + (g + 1) * P, :] = \
                rows[sb * P:(sb + 1) * P]
    return out


# revision 125
# speedup vs baseline: 1.0933x; 1.0933x over previous
"""Trainium2 Bass kernel: pre-LN transformer decoder layer on 8 NeuronCores.

Sharding: core = 4*b + g  (b in {0,1} batch, g in {0..3} group rank).
  - Attention: head-parallel (4 of 16 heads per core) over the full batch-b
    sequence, computed in a TRANSPOSED-score formulation:
      scores_T[k, q] = K_blk @ Q^T   (softmax needs no max-subtraction here;
      scores are O(6)), exp on ScalarE with the key-padding mask as a
      per-partition bias, denominator via an appended ones-column on V,
      so no P-transposes and no reductions at all.  attn output comes out
      already transposed [dh, q] — perfect for the output projection.
  - Output projection partials (4 local heads) are computed per 512-token
    group and exchanged with chunked bf16 ReduceScatters (1 MB each),
    overlapped with the attention compute of later token groups.
  - FFN: token-parallel (512 tokens per core, strided 128-blocks) with
    full weights (bf16).
All matmul operands are bf16 (fp32 accumulation in PSUM).
"""
import math
import numpy as np
import ml_dtypes

import concourse.bacc as bacc
import concourse.bass as bass
import concourse.tile as tile
from concourse import mybir
from concourse.masks import make_identity

B, S, D, H, DH, DFF = 2, 2048, 1024, 16, 64, 4096
G = 4            # cores per batch
LH = H // G      # local heads
LD = LH * DH     # 256 local head dims
SL = S // G      # 512 tokens per core for FFN / output projection
P = 128
NB = S // P      # 16 token blocks
DC = D // P      # 8 d chunks
F32 = mybir.dt.float32
F32R = mybir.dt.float32r
BF16 = mybir.dt.bfloat16
NEG = -1e9
NPBF16 = ml_dtypes.bfloat16

_CACHE = {}


def r(ap):
    return ap.bitcast(F32R)


def build_nc():
    nc = bacc.Bacc("TRN2", target_bir_lowering=False, debug=False, num_devices=8)
    d = {}
    def inp(name, shape, dt=F32):
        d[name] = nc.dram_tensor(name, list(shape), dt, kind="ExternalInput").ap()
    inp("xfull", (S, D), BF16)
    inp("xrows", (SL, D))
    inp("wqT", (D, LD), BF16); inp("wkT", (D, LD), BF16); inp("wvT", (D, LD), BF16)
    inp("qb", (P, 2)); inp("kb2", (P, 2)); inp("vb_bc", (P, LD))
    inp("wo_pair", (P, 2, D), BF16); inp("wob_bc", (P, D))
    inp("mdiagT", (P, P)); inp("rmask16", (P, NB)); inp("qm16", (P, NB))

    inp("w1T", (D, DFF), BF16); inp("b1p", (P, DFF // P))
    inp("w2T", (DFF, D), BF16); inp("b2_bc", (P, D))
    out_rows = nc.dram_tensor("out_rows", [SL, D], F32, kind="ExternalOutput").ap()
    partial_d = nc.dram_tensor("partial_d", [S, D], BF16).ap()
    rs_d = nc.dram_tensor("rs_d", [SL, D], BF16).ap()

    with tile.TileContext(nc) as tc:
        with tc.tile_pool(name="consts", bufs=1) as consts:
            identb = consts.tile([P, P], BF16)
            make_identity(nc, identb)
            eps_sb = consts.tile([P, 1], F32)
            nc.vector.memset(eps_sb, 1e-5)
            w1_cm = tc.tile_pool(name="w1p", bufs=1)
            w1_pool = w1_cm.__enter__()
            w1a = w1_pool.tile([P, DC, DFF // 2], BF16)   # first half prefetched
            pw_cm = tc.tile_pool(name="pw", bufs=1)
            pw = pw_cm.__enter__()
            wo_sb = pw.tile([P, 2, D], BF16)
            mdT = pw.tile([P, P], F32)
            rm_sb = pw.tile([P, NB], F32)
            xr_sb = pw.tile([P, 4, D], F32)
            wob = pw.tile([P, D], F32)
            b1_sb = pw.tile([P, DFF // P], F32)
            b2_sb = pw.tile([P, D], F32)
            XP = pw.tile([P, 4, D], F32)   # X' rows (post-attn residual)
            YNT = pw.tile([P, DC, SL], BF16)
            dpst_cm = tc.tile_pool(name="d_ps_t", bufs=2, space="PSUM")
            d_ps_t = dpst_cm.__enter__()
            qt_cm = tc.tile_pool(name="qt", bufs=1)
            qt_pool = qt_cm.__enter__()
            QT = qt_pool.tile([P, 2, S], BF16)      # [dh-in-pair, pb, s] (q scaled+bias)
            KT = qt_pool.tile([P, 2, S], BF16)
            V = qt_pool.tile([P, NB, LH, DH + 1], BF16)   # token-major V + ones col
            nc.vector.memset(V[:, :, :, DH:DH + 1], 1.0)

            # ---------------- Phase A: LN1 + transpose, Phase B: QKV ----------
            with tc.tile_pool(name="ab", bufs=4) as ab, \
                 tc.tile_pool(name="abw", bufs=1) as abw, \
                 tc.tile_pool(name="xnt_p", bufs=1) as xnt_p, \
                 tc.tile_pool(name="qkps", bufs=2, space="PSUM") as qkps:
                wq_sb = abw.tile([P, DC, LD], BF16)
                wk_sb = abw.tile([P, DC, LD], BF16)
                wv_sb = abw.tile([P, DC, LD], BF16)
                qb_sb = abw.tile([P, 2], F32); nc.sync.dma_start(out=qb_sb, in_=d["qb"][:])
                kb_sb = abw.tile([P, 2], F32); nc.sync.dma_start(out=kb_sb, in_=d["kb2"][:])
                vb_sb = abw.tile([P, LD], F32); nc.sync.dma_start(out=vb_sb, in_=d["vb_bc"][:])
                qm_sb = abw.tile([P, NB], F32); nc.sync.dma_start(out=qm_sb, in_=d["qm16"][:])
                XNT = xnt_p.tile([P, DC, S], BF16)

                for i in range(NB):
                    xin = ab.tile([P, D], BF16, tag="xin")
                    nc.sync.dma_start(out=xin, in_=d["xfull"][i * P:(i + 1) * P, :])
                    stats = ab.tile([P, 2, 6], F32, tag="st")
                    nc.vector.bn_stats(out=stats[:, 0, :], in_=xin[:, 0:512])
                    nc.vector.bn_stats(out=stats[:, 1, :], in_=xin[:, 512:1024])
                    mv = ab.tile([P, 2], F32, tag="mv")
                    nc.vector.bn_aggr(out=mv, in_=stats)
                    rs_t = ab.tile([P, 1], F32, tag="rs")
                    nc.scalar.activation(out=rs_t, in_=mv[:, 1:2],
                                         func=mybir.ActivationFunctionType.Sqrt,
                                         bias=eps_sb)
                    nc.vector.reciprocal(out=rs_t, in_=rs_t)
                    # zero padded-token rows of Xn: their q columns become
                    # exactly 0 (folded q-bias is 0 too), so exp gives uniform
                    # weights over valid causal keys — matching the reference's
                    # fp32-quantized -1e9 masking on padded query rows
                    nc.vector.tensor_tensor(out=rs_t, in0=rs_t,
                                            in1=qm_sb[:, i:i + 1],
                                            op=mybir.AluOpType.mult)
                    nmurs = ab.tile([P, 1], F32, tag="nm")
                    nc.vector.tensor_tensor(out=nmurs, in0=mv[:, 0:1], in1=rs_t,
                                            op=mybir.AluOpType.mult)
                    nc.vector.tensor_scalar_mul(nmurs, nmurs, -1.0)
                    xnb = ab.tile([P, D], BF16, tag="xnb")
                    nc.scalar.activation(out=xnb, in_=xin,
                                         func=mybir.ActivationFunctionType.Identity,
                                         scale=rs_t, bias=nmurs)
                    for j in range(DC // 2):
                        pt = d_ps_t.tile([P, 2, P], BF16, tag="tp3")
                        nc.tensor.transpose(pt[:, 0, :], xnb[:, 2 * j * P:(2 * j + 1) * P], identb)
                        nc.tensor.transpose(pt[:, 1, :], xnb[:, (2 * j + 1) * P:(2 * j + 2) * P], identb)
                        nc.any.tensor_copy(out=XNT[:, 2 * j:2 * j + 2, i * P:(i + 1) * P],
                                           in_=pt)

                # QKV weights + FFN w1 + all later-phase weights load here, after
                # the input-stream DMAs are issued, so they never delay startup
                nc.sync.dma_start(out=wq_sb, in_=d["wqT"].rearrange("(c p) o -> p c o", p=P))
                nc.sync.dma_start(out=wk_sb, in_=d["wkT"].rearrange("(c p) o -> p c o", p=P))
                nc.sync.dma_start(out=wv_sb, in_=d["wvT"].rearrange("(c p) o -> p c o", p=P))
                w1rr = d["w1T"].rearrange("(c p) m -> p c m", p=P)
                for c in range(DC):
                    nc.sync.dma_start(out=w1a[:, c, :], in_=w1rr[:, c, 0:DFF // 2])
                nc.sync.dma_start(out=wo_sb, in_=d["wo_pair"][:])
                nc.sync.dma_start(out=mdT, in_=d["mdiagT"][:])
                nc.sync.dma_start(out=rm_sb, in_=d["rmask16"][:])
                nc.sync.dma_start(out=xr_sb, in_=d["xrows"].rearrange("(s p) d -> p s d", p=P))
                nc.sync.dma_start(out=wob, in_=d["wob_bc"][:])
                nc.sync.dma_start(out=b1_sb, in_=d["b1p"][:])
                nc.sync.dma_start(out=b2_sb, in_=d["b2_bc"][:])

                # QKV projections
                for pb in range(2):
                    for sc in range(S // 512):
                        psq = qkps.tile([P, 512], F32, tag="psq")
                        psk = qkps.tile([P, 512], F32, tag="psk")
                        for dc in range(DC):
                            nc.tensor.matmul(psq, wq_sb[:, dc, pb * P:(pb + 1) * P],
                                             XNT[:, dc, sc * 512:(sc + 1) * 512],
                                             start=(dc == 0), stop=(dc == DC - 1))
                        for dc in range(DC):
                            nc.tensor.matmul(psk, wk_sb[:, dc, pb * P:(pb + 1) * P],
                                             XNT[:, dc, sc * 512:(sc + 1) * 512],
                                             start=(dc == 0), stop=(dc == DC - 1))
                        nc.scalar.activation(out=QT[:, pb, sc * 512:(sc + 1) * 512],
                                             in_=psq,
                                             func=mybir.ActivationFunctionType.Identity,
                                             bias=qb_sb[:, pb:pb + 1])

                        nc.scalar.activation(out=KT[:, pb, sc * 512:(sc + 1) * 512],
                                             in_=psk,
                                             func=mybir.ActivationFunctionType.Identity,
                                             bias=kb_sb[:, pb:pb + 1])
                for sb in range(NB):
                    psv = qkps.tile([P, LD], F32, tag="psv")
                    for dc in range(DC):
                        nc.tensor.matmul(psv, XNT[:, dc, sb * P:(sb + 1) * P],
                                         wv_sb[:, dc, :],
                                         start=(dc == 0), stop=(dc == DC - 1))
                    nc.vector.tensor_tensor(
                        out=V[:, sb, :, 0:DH],
                        in0=psv[:].rearrange("p (h e) -> p h e", h=LH),
                        in1=vb_sb[:].rearrange("p (h e) -> p h e", h=LH),
                        op=mybir.AluOpType.add)

            # ---------------- Phase C: attention (transposed scores) ----------
            with tc.tile_pool(name="cat", bufs=2) as cat, \
                 tc.tile_pool(name="cee", bufs=3) as cee, \
                 tc.tile_pool(name="csm", bufs=2) as csm, \
                 tc.tile_pool(name="c_ps_s", bufs=2, space="PSUM") as c_ps_s, \
                 tc.tile_pool(name="c_ps_at", bufs=1, space="PSUM") as c_ps_at:
                ATgs = {}
                ynbs = {}

                def emit_wo_partials(g):
                    # output-projection partials for token group g (bf16) + RS
                    for qi2 in range(4):
                        row0 = (g * 4 + qi2) * P
                        for oc in range(2):
                            pp = c_ps_s.tile([P, 512], F32, tag="ps0", name="pp")
                            for hp in range(2):
                                nc.tensor.matmul(
                                    pp, ATgs[g][:, hp, qi2 * P:(qi2 + 1) * P],
                                    wo_sb[:, hp, oc * 512:(oc + 1) * 512],
                                    start=(hp == 0), stop=(hp == 1))
                            ppb = cee.tile([P, 512], BF16, tag="ppb")
                            nc.any.tensor_copy(out=ppb, in_=pp)
                            nc.sync.dma_start(
                                out=partial_d[row0:row0 + P, oc * 512:(oc + 1) * 512],
                                in_=ppb)
                    nc.gpsimd.collective_compute(
                        "ReduceScatter", mybir.AluOpType.add,
                        replica_groups=[[0, 1, 2, 3], [4, 5, 6, 7]],
                        ins=[partial_d[g * 512:(g + 1) * 512, :]],
                        outs=[rs_d[g * P:(g + 1) * P, :]])

                def emit_d_pro_vec(sb):
                    # residual + LN2 for token block sb (DVE/ScalarE only)
                    rs_sb = csm.tile([P, D], BF16, tag="rs_in")
                    nc.sync.dma_start(out=rs_sb, in_=rs_d[sb * P:(sb + 1) * P, :])
                    nc.vector.tensor_tensor(out=XP[:, sb, :], in0=rs_sb,
                                            in1=xr_sb[:, sb, :],
                                            op=mybir.AluOpType.add)
                    nc.vector.tensor_tensor(out=XP[:, sb, :], in0=XP[:, sb, :],
                                            in1=wob, op=mybir.AluOpType.add)
                    stats = csm.tile([P, 2, 6], F32, tag="st2")
                    nc.vector.bn_stats(out=stats[:, 0, :], in_=XP[:, sb, 0:512])
                    nc.vector.bn_stats(out=stats[:, 1, :], in_=XP[:, sb, 512:1024])
                    mv = csm.tile([P, 2], F32, tag="mv2")
                    nc.vector.bn_aggr(out=mv, in_=stats)
                    rs_t = csm.tile([P, 1], F32, tag="rs2")
                    nc.scalar.activation(out=rs_t, in_=mv[:, 1:2],
                                         func=mybir.ActivationFunctionType.Sqrt,
                                         bias=eps_sb)
                    nc.vector.reciprocal(out=rs_t, in_=rs_t)
                    nmurs = csm.tile([P, 1], F32, tag="nm2")
                    nc.vector.tensor_tensor(out=nmurs, in0=mv[:, 0:1], in1=rs_t,
                                            op=mybir.AluOpType.mult)
                    nc.vector.tensor_scalar_mul(nmurs, nmurs, -1.0)
                    ynb = ynbs[sb] = pw.tile([P, D], BF16, tag=f"ynb{sb}",
                                             name="ynb")
                    nc.scalar.activation(out=ynb, in_=XP[:, sb, :],
                                         func=mybir.ActivationFunctionType.Identity,
                                         scale=rs_t, bias=nmurs)

                def emit_d_pro_tp(sb):
                    # transpose ynb into YNT (tensor engine)
                    for j in range(DC // 2):
                        tp = d_ps_t.tile([P, 2, P], BF16, tag="tp3")
                        nc.tensor.transpose(tp[:, 0, :],
                                            ynbs[sb][:, 2 * j * P:(2 * j + 1) * P],
                                            identb)
                        nc.tensor.transpose(tp[:, 1, :],
                                            ynbs[sb][:, (2 * j + 1) * P:(2 * j + 2) * P],
                                            identb)
                        nc.any.tensor_copy(
                            out=YNT[:, 2 * j:2 * j + 2, sb * P:(sb + 1) * P],
                            in_=tp)

                for gq in range(4):
                    ATg = cat.tile([P, 2, 512], BF16, tag="ATg")
                    ATgs[gq] = ATg
                    for pb in range(2):   # head pair (2*pb, 2*pb+1), row-packed
                        if pb == 1 and gq > 0:
                            # previous group's partials: its ATg is long since
                            # ready, so these don't stall the tensor queue
                            emit_wo_partials(gq - 1)
                        if gq == 3:
                            # residual+LN2 for early token blocks (DVE-only;
                            # their RS chunks completed a group ago)
                            emit_d_pro_vec(pb)
                        at0 = c_ps_at.tile([DH + 1, 512], F32, tag="at0")
                        at1 = c_ps_at.tile([DH + 1, 512], F32, tag="at1")
                        ats = (at0, at1)
                        for kb in range(4 * gq + 4):
                            q0, qe = max(kb, 4 * gq), 4 * gq + 4
                            w = (qe - q0) * P
                            col0 = (q0 - 4 * gq) * P
                            ps0 = c_ps_s.tile([P, 512], F32, tag="ps0")
                            ps1 = c_ps_s.tile([P, 512], F32, tag="ps1")
                            pss = (ps0, ps1)
                            # two heads in different PE row-groups -> concurrent
                            for i, po in enumerate((0, 64)):
                                nc.tensor.matmul(
                                    pss[i][:, :w],
                                    KT[po:po + 64, pb, kb * P:(kb + 1) * P],
                                    QT[po:po + 64, pb, q0 * P:qe * P],
                                    start=True, stop=True)
                            for i in range(2):
                                if q0 == kb:
                                    nc.vector.tensor_tensor(out=pss[i][:, 0:P],
                                                            in0=pss[i][:, 0:P],
                                                            in1=mdT,
                                                            op=mybir.AluOpType.add)
                                ept = cee.tile([P, 512], BF16, tag="ept")
                                nc.scalar.activation(
                                    out=ept[:, :w], in_=pss[i][:, :w],
                                    func=mybir.ActivationFunctionType.Exp,
                                    bias=rm_sb[:, kb:kb + 1])
                                Vst = V[:, kb, 2 * pb + i, :]
                                if q0 == kb:
                                    nc.tensor.matmul(ats[i][:, col0:col0 + P], Vst,
                                                     ept[:, 0:P],
                                                     start=(kb == 0), stop=True)
                                    if w > P:
                                        nc.tensor.matmul(ats[i][:, col0 + P:col0 + w],
                                                         Vst, ept[:, P:w],
                                                         start=(kb == 0), stop=False)
                                else:
                                    nc.tensor.matmul(ats[i][:, 0:w], Vst, ept[:, :w],
                                                     start=(kb == 0), stop=False)
                        for i in range(2):
                            rinv = csm.tile([1, 512], F32, tag="ri")
                            nc.vector.reciprocal(out=rinv, in_=ats[i][DH:DH + 1, :])
                            rb_sb = csm.tile([64, 512], F32, tag="rbs")
                            nc.gpsimd.partition_broadcast(rb_sb, rinv[0:1, :],
                                                          channels=64)
                            nc.vector.tensor_tensor(out=ATg[i * 64:(i + 1) * 64, pb, :],
                                                    in0=ats[i][0:DH, :], in1=rb_sb,
                                                    op=mybir.AluOpType.mult)
                emit_wo_partials(3)
                emit_d_pro_tp(0)
                emit_d_pro_tp(1)
                emit_d_pro_vec(2)
                emit_d_pro_tp(2)
                emit_d_pro_vec(3)

            qt_cm.__exit__(None, None, None)

            # ---------------- Phase D: FFN (LN2 prologue already emitted) -----
            with tc.tile_pool(name="dh", bufs=1) as dh_p, \
                 tc.tile_pool(name="dw1", bufs=3) as dw1, \
                 tc.tile_pool(name="dw2", bufs=6) as dw2_p, \
                 tc.tile_pool(name="dfin", bufs=3) as dfin, \
                 tc.tile_pool(name="d_ps_h", bufs=2, space="PSUM") as d_ps_h, \
                 tc.tile_pool(name="d_ps_o", bufs=1, space="PSUM") as d_ps_o:
                HT = dh_p.tile([P, DFF // P, SL], BF16)

                def w1_half(th):
                    # first token half (sb 0,1) starts while the last RS chunk
                    # is still in flight; upper-half w1 chunks are streamed
                    t0, t1 = th * 256, (th + 1) * 256
                    for c in range(DFF // P):
                        if c < DFF // (2 * P):
                            w1c = [w1a[:, dc, c * P:(c + 1) * P] for dc in range(DC)]
                        else:
                            w1t = dw1.tile([P, DC, P], BF16, tag="w1c")
                            nc.sync.dma_start(out=w1t,
                                              in_=w1rr[:, :, c * P:(c + 1) * P])
                            w1c = [w1t[:, dc, :] for dc in range(DC)]
                        ps_h = d_ps_h.tile([P, 256], F32, tag="ps_h")
                        for dc in range(DC):
                            nc.tensor.matmul(ps_h, w1c[dc], YNT[:, dc, t0:t1],
                                             start=(dc == 0), stop=(dc == DC - 1))
                        nc.scalar.activation(out=HT[:, c, t0:t1], in_=ps_h,
                                             func=mybir.ActivationFunctionType.Gelu,
                                             bias=b1_sb[:, c:c + 1])

                w1_half(0)
                emit_d_pro_tp(3)
                w1_half(1)
                w2r = d["w2T"].rearrange("(c p) o -> p c o", p=P)

                for oc in range(2):
                    ps_os = [d_ps_o.tile([P, 512], F32, tag=f"ps_o{sb}", name=f"ps_o{sb}")
                             for sb in range(4)]
                    for c in range(DFF // P):
                        w2c = dw2_p.tile([P, 512], BF16, tag="w2c")
                        nc.sync.dma_start(out=w2c,
                                          in_=w2r[:, c, oc * 512:(oc + 1) * 512])
                        for sb in range(4):
                            nc.tensor.matmul(
                                ps_os[sb], HT[:, c, sb * P:(sb + 1) * P],
                                w2c,
                                start=(c == 0), stop=(c == DFF // P - 1))
                    for sb in range(4):
                        fin = dfin.tile([P, 512], F32, tag="fin")
                        nc.vector.tensor_tensor(out=fin, in0=ps_os[sb],
                                                in1=b2_sb[:, oc * 512:(oc + 1) * 512],
                                                op=mybir.AluOpType.add)
                        nc.vector.tensor_tensor(out=fin, in0=fin,
                                                in1=XP[:, sb, oc * 512:(oc + 1) * 512],
                                                op=mybir.AluOpType.add)
                        nc.sync.dma_start(
                            out=out_rows[sb * P:(sb + 1) * P, oc * 512:(oc + 1) * 512],
                            in_=fin)

            dpst_cm.__exit__(None, None, None)
            pw_cm.__exit__(None, None, None)
            w1_cm.__exit__(None, None, None)

    nc.compile()
    return nc


def make_in_maps(X, mask, valid_lens, wq_w, wq_b, wk_w, wv_w, wv_b, wo_w, wo_b,
                 ln1_g, ln1_b, ln2_g, ln2_b, w1, b1, w2, b2):
    f = np.float32
    bc = lambda v: np.broadcast_to(np.asarray(v, f)[None, :], (P, len(v))).copy()
    # transposed causal diagonal block (identical for every kb):
    # mdT[kk, qq] = mask[qq, kk] for the block-local causal pattern
    mdT = np.ascontiguousarray(np.asarray(mask[0:P, 0:P]).T).astype(f)
    idx = np.arange(S)
    woT = np.ascontiguousarray(np.asarray(wo_w, f).T)   # [d_in, d_out]
    p_ar = np.arange(P)
    in_maps = []
    for core in range(8):
        b, g = core // G, core % G
        vmask1 = np.where(idx >= valid_lens[b], NEG, 0.0).astype(f)
        hs = slice(g * LD, (g + 1) * LD)
        # local-head-pair layout matching ATg: partition p = (h%2)*64 + dh,
        # second dim hp = h//2, local head h = hp*2 + p//64
        wo_pair = np.empty((P, 2, D), NPBF16)
        for hp in range(2):
            head = g * 4 + hp * 2 + p_ar // 64
            rows = head * 64 + (p_ar % 64)
            wo_pair[:, hp, :] = woT[rows, :].astype(NPBF16)
        # strided token ownership: rows sb*512 + g*128 .. +128, sb = 0..3
        own = np.concatenate([np.arange(sb * 512 + g * P, sb * 512 + (g + 1) * P)
                              for sb in range(4)])
        m = {
            "xfull": np.ascontiguousarray(X[b]).astype(NPBF16),
            "xrows": np.ascontiguousarray(X[b][own]).astype(f),
            "wqT": np.ascontiguousarray(
                (np.asarray(wq_w, f)[hs] * np.asarray(ln1_g, f)[None, :] * 0.125)
                .T).astype(NPBF16),
            "wkT": np.ascontiguousarray(
                (np.asarray(wk_w, f)[hs] * np.asarray(ln1_g, f)[None, :])
                .T).astype(NPBF16),
            "wvT": np.ascontiguousarray(
                (np.asarray(wv_w, f)[hs] * np.asarray(ln1_g, f)[None, :])
                .T).astype(NPBF16),
            "qb": np.ascontiguousarray(
                (0.125 * (wq_b[hs] + ln1_b @ np.asarray(wq_w, f)[hs].T))
                .reshape(2, P).T).astype(f),
            "kb2": np.ascontiguousarray(
                (ln1_b @ np.asarray(wk_w, f)[hs].T).reshape(2, P).T).astype(f),
            "vb_bc": bc(wv_b[hs] + ln1_b @ np.asarray(wv_w, f)[hs].T),
            "wo_pair": wo_pair,
            "wob_bc": bc(wo_b),
            "mdiagT": mdT,
            "rmask16": np.ascontiguousarray(vmask1.reshape(NB, P).T).astype(f),
            "qm16": np.ascontiguousarray(
                np.where(idx < valid_lens[b], 1.0, 0.0).astype(f)
                .reshape(NB, P).T).astype(f),
            "w1T": np.ascontiguousarray(
                (np.asarray(w1, f) * np.asarray(ln2_g, f)[None, :])
                .T).astype(NPBF16),
            "b1p": np.ascontiguousarray(
                np.asarray(b1 + ln2_b @ np.asarray(w1, f).T, f)
                .reshape(DFF // P, P).T).astype(f),
            "w2T": np.ascontiguousarray(np.asarray(w2, f).T).astype(NPBF16),
            "b2_bc": bc(b2),
        }
        in_maps.append(m)
    return in_maps


def kernel(**inputs):
    from concourse.bass_utils import run_bass_kernel_spmd
    if "nc" not in _CACHE:
        _CACHE["nc"] = build_nc()
    nc = _CACHE["nc"]
    in_maps = make_in_maps(**inputs)
    res = run_bass_kernel_spmd(nc, in_maps, list(range(8)))
    out = np.empty((B, S, D), np.float32)
    for core in range(8):
        b, g = core // G, core % G
        rows = res.results[core]["out_rows"]
        for sb in range(4):
            out[b, sb * 512 + g * P:sb * 512 + (g + 1) * P, :] = \
                rows[sb * P:(sb + 1) * P]
    return out
